# revision 12
# baseline (speedup 1.0000x reference)
"""Trainium2 Bass kernel for the DataReloadingQNN problem.

Math: layers 0..4 and the shared layer-5 gates B_q = RZ RY RZ are
sample-independent -> shared state v5.  The per-sample part is
    state_b = P . prod_q RY_q(x_bq) . v5          (P = CNOT chain)
RY_q = c_q I + s_q J_q.  Expanding qubits 2..10 gives
    t_b = sum_{m<512} W[b,m] u_m                  (matmul, K=512)
with u_m = J^m v5 and the CNOT permutation P folded into columns.
Qubits 0 and 1 are applied after the matmul as per-sample butterflies;
P is GF(2)-linear, so in P-space the qubit-1 pairing is a ^ 1023 and
the qubit-0 pairing is a ^ 2047.  U columns are stored de-interleaved
(re plane then im plane) and chunks 1,3 of each plane are stored
column-REVERSED, which turns both pairings into same-index chunk swaps:
    q1: chunk c <-> c^1 (within 1024-halves), signs [-,+,+,-]
    q0: chunk c <-> c^3 (across halves),      signs [-,-,+,+]
Signs are folded into constant +-1 tiles (sig1 = [+,-,-,+],
sig0 = [+,+,-,-]), so each butterfly is two wide contiguous
scalar_tensor_tensor ops.  The host un-reverses chunks 1,3 at the end.
Device work per core (1024 samples):
  1. cos/sin of x/2 on ScalarE
  2. W (128 x 512 per sample-tile): tiny doublings + one broadcast
     outer product on VectorE; PE transpose (ScalarE copies out)
  3. t = W @ U as bf16 matmuls, K=512 (PSUM half-tiles of 2 banks)
  4. butterflies on VectorE (bf16 intermediates), DMA out bf16
Inputs are sharded batch-wise across 8 cores; U (params-derived)
replicated.  Host converts bf16 -> f32 and reassembles (B, 2048, 2).
"""
import numpy as np
import ml_dtypes

import concourse.bass as bass
import concourse.bacc as bacc
import concourse.tile as tile
from concourse import mybir
from concourse.bass_utils import run_bass_kernel_spmd

N = 11
DIM = 2048
BATCH = 8192
NCORES = 8
BSH = BATCH // NCORES          # 1024 samples per core
NTILES = BSH // 128            # 8 sample-tiles per core
NPULL = 2                      # qubits 0,1 pulled out of the expansion
K = 512                        # contraction dim (qubits 2..10 expanded)
KT = K // 128                  # 4 k-tiles
NW = 512                       # columns per storage chunk
F32 = mybir.dt.float32
BF16 = mybir.dt.bfloat16

# storage permutation: chunks 1,3 of each plane column-reversed
IDX = np.r_[0:512, np.arange(1023, 511, -1), 1024:1536,
            np.arange(2047, 1535, -1)]

# ---------------------------------------------------------------- host math

def _rz(phi):
    e = np.exp(-0.5j * phi)
    return np.array([[e, 0], [0, np.conj(e)]], dtype=np.complex128)


def _ry(theta):
    t = 0.5 * theta
    c, s = np.cos(t), np.sin(t)
    return np.array([[c, -s], [s, c]], dtype=np.complex128)


def _apply_1q_rows(rows, U, q):
    R = rows.shape[0]
    st = rows.reshape(R, 2 ** q, 2, 2 ** (N - 1 - q))
    st = np.einsum('ab,rxby->rxay', U, st)
    return st.reshape(R, DIM)


def _apply_cnot_rows(rows, c):
    R = rows.shape[0]
    st = rows.reshape(R, 2 ** c, 2, 2, 2 ** (N - 2 - c))
    st = np.stack([st[:, :, 0], st[:, :, 1, ::-1]], axis=2)
    return st.reshape(R, DIM)


def build_u_matrix(params):
    """(6,11,3) f32 -> U (512, 4096) f64 in device storage order."""
    p = params.astype(np.float64)
    v = np.zeros((1, DIM), dtype=np.complex128)
    v[0, 0] = 1.0
    for l in range(5):
        for q in range(N):
            v = _apply_1q_rows(v, _rz(p[l, q, 0]), q)
            v = _apply_1q_rows(v, _ry(p[l, q, 1]), q)
            v = _apply_1q_rows(v, _rz(p[l, q, 2]), q)
        for c in range(N - 1):
            v = _apply_cnot_rows(v, c)
    for q in range(N):
        B = _rz(p[5, q, 2]) @ _ry(p[5, q, 1]) @ _rz(p[5, q, 0])
        v = _apply_1q_rows(v, B, q)

    J = np.array([[0, -1], [1, 0]], dtype=np.complex128)
    rows = v
    for q in range(NPULL, N):      # expand qubits 2..10; bit i <-> qubit i+2
        rc = _apply_1q_rows(rows, J, q)
        rows = np.concatenate([rows, rc], axis=0)

    g = np.arange(DIM)[None, :]
    for c in range(N - 1):
        g = _apply_cnot_rows(g.astype(np.float64), c).astype(np.int64)
    rows = rows[:, g[0]]           # fold CNOT permutation
    rows = rows[:, IDX]            # storage order (chunks 1,3 reversed)

    # joint-plane layout: col block c (1024 wide) = [re chunk c | im chunk c]
    U = np.empty((K, 2 * DIM), dtype=np.float64)
    for c in range(4):
        U[:, 1024 * c:1024 * c + 512] = rows.real[:, 512 * c:512 * (c + 1)]
        U[:, 1024 * c + 512:1024 * (c + 1)] = rows.imag[:, 512 * c:512 * (c + 1)]
    return U

# ------------------------------------------------------------- bass kernel

def build_kernel():
    nc = bacc.Bacc()
    x_d = nc.dram_tensor("x", (BSH, N), F32, kind="ExternalInput")
    u_d = nc.dram_tensor("u", (KT, 128, 2 * DIM), BF16, kind="ExternalInput")
    id_d = nc.dram_tensor("ident", (128, 128), BF16, kind="ExternalInput")
    out_d = nc.dram_tensor("out", (BSH, 2 * DIM), BF16, kind="ExternalOutput")

    MULT = mybir.AluOpType.mult
    ADD = mybir.AluOpType.add
    SUB = mybir.AluOpType.subtract

    NQLO, NQHI = 5, 4            # qubits 2..6 -> low bits, 7..10 -> high
    WLO, WHI = 1 << NQLO, 1 << NQHI

    with tile.TileContext(nc) as tc:
        with (
            tc.tile_pool(name="const", bufs=1) as const_pool,
            tc.tile_pool(name="wbuild", bufs=2) as wbuild_pool,
            tc.tile_pool(name="wt", bufs=1) as wt_pool,
            tc.tile_pool(name="uin", bufs=1) as u_pool,
            tc.tile_pool(name="ys", bufs=3) as y_pool,
            tc.tile_pool(name="outs", bufs=3) as out_pool,
            tc.tile_pool(name="tmps", bufs=3) as tmp_pool,
        ):
            ident = const_pool.tile([128, 128], BF16)
            nc.sync.dma_start(ident[:], id_d[:])

            # x: (1024, 11) -> sbuf (128, 8*11); sample-tile t in cols
            # [t*11, (t+1)*11)
            x_sb = const_pool.tile([128, NTILES * N], F32)
            x_r = x_d.rearrange("(t p) f -> p t f", p=128)
            nc.sync.dma_start(x_sb[:].rearrange("p (t f) -> p t f", f=N), x_r)

            cos_sb = const_pool.tile([128, NTILES * N], F32)
            sin_sb = const_pool.tile([128, NTILES * N], F32)
            hp_t = const_pool.tile([128, 1], F32)
            zr_t = const_pool.tile([128, 1], F32)
            nc.vector.memset(hp_t[:], float(np.pi / 2))
            nc.vector.memset(zr_t[:], 0.0)
            # dummy Sin to pull the ACT table load off the critical path
            nc.scalar.activation(hp_t[:], zr_t[:],
                                 mybir.ActivationFunctionType.Sin,
                                 bias=zr_t[:], scale=1.0)
            nc.vector.memset(hp_t[:], float(np.pi / 2))
            # cos(t) = sin(pi/2 - t): keeps Sin args in (-pi/2, pi/2]
            nc.scalar.activation(cos_sb[:], x_sb[:],
                                 mybir.ActivationFunctionType.Sin,
                                 bias=hp_t[:], scale=-0.5)
            nc.scalar.activation(sin_sb[:], x_sb[:],
                                 mybir.ActivationFunctionType.Sin,
                                 bias=zr_t[:], scale=0.5)

            # all of U up front: one 3D DMA per joint chunk (re|im, 1024
            # cols), spread over the two HWDGE queues
            uts = []
            for c in range(4):
                ut = u_pool.tile([128, KT * 2 * NW], BF16, tag=f"u{c}")
                eng = nc.sync if c % 2 == 0 else nc.scalar
                eng.dma_start(
                    ut[:].rearrange("p (k j) -> p k j", j=2 * NW),
                    u_d[:, :, 2 * NW * c:2 * NW * (c + 1)]
                    .rearrange("k p j -> p k j"))
                uts.append(ut)

            # cos/sin pairs interleaved: cs[:, 2*(t*N+q)] = cos, +1 = sin
            cs = const_pool.tile([128, NTILES * N * 2], F32)
            csv = cs[:].rearrange("p (q c) -> p q c", c=2)
            nc.vector.tensor_copy(csv[:, :, 0:1],
                                  cos_sb[:].rearrange("p (q u) -> p q u", u=1))
            nc.vector.tensor_copy(csv[:, :, 1:2],
                                  sin_sb[:].rearrange("p (q u) -> p q u", u=1))

            def double_step(nxt, cur, w, off):
                """nxt[:, 0:2w] = [cur*cos_q | cur*sin_q] in one op."""
                ov = nxt[:, 0:2 * w].rearrange("p (c w) -> p c w", w=w)
                i0 = cur[:, 0:w].rearrange("p (u w) -> p u w", u=1) \
                    .broadcast_to((128, 2, w))
                i1 = cs[:, off:off + 2].rearrange("p (c u) -> p c u", u=1) \
                    .broadcast_to((128, 2, w))
                nc.vector.tensor_tensor(ov, i0, i1, MULT)

            # Phase A: W per sample-tile = outer(wHigh, wLow), bit i of the
            # expansion <-> qubit i+2 (low bits = qubits 2..6)
            wts = []
            with tc.tile_pool(name="ptr", bufs=2,
                              space=bass.MemorySpace.PSUM) as ptr_pool:
                for t in range(NTILES):
                    col = t * N + NPULL
                    wlo = wbuild_pool.tile([128, WLO], F32, tag="wlo")
                    wlob = wbuild_pool.tile([128, WLO], F32, tag="wlob")
                    nc.vector.tensor_copy(wlo[:, 0:2], cs[:, 2 * col:2 * col + 2])
                    cur, nxt = wlo, wlob
                    for j in range(1, NQLO):
                        double_step(nxt, cur, 1 << j, 2 * (col + j))
                        cur, nxt = nxt, cur
                    wlo_f = cur

                    colh = col + NQLO
                    whi = wbuild_pool.tile([128, WHI], F32, tag="whi")
                    whib = wbuild_pool.tile([128, WHI], F32, tag="whib")
                    nc.vector.tensor_copy(whi[:, 0:2],
                                          cs[:, 2 * colh:2 * colh + 2])
                    cur, nxt = whi, whib
                    for j in range(1, NQHI):
                        double_step(nxt, cur, 1 << j, 2 * (colh + j))
                        cur, nxt = nxt, cur
                    whi_f = cur

                    # W[b, i*WLO + j] = wHigh[b,i] * wLow[b,j], bf16
                    wbf = wbuild_pool.tile([128, K], BF16, tag="wbf")
                    av = whi_f[:].rearrange("p (i u) -> p i u", u=1) \
                        .broadcast_to((128, WHI, WLO))
                    bv = wlo_f[:].rearrange("p (u j) -> p u j", u=1) \
                        .broadcast_to((128, WHI, WLO))
                    ov = wbf[:].rearrange("p (i j) -> p i j", j=WLO)
                    nc.vector.tensor_tensor(ov, av, bv, MULT)

                    wt = wt_pool.tile([128, KT * 128], BF16, tag=f"wt{t}")
                    for k in range(KT):
                        ptr = ptr_pool.tile([128, 128], BF16)
                        nc.tensor.transpose(
                            ptr[:], wbf[:, k * 128:(k + 1) * 128], ident[:])
                        nc.vector.tensor_copy(wt[:, k * 128:(k + 1) * 128], ptr[:])
                    wts.append(wt)

            # Phase B per tile: two psum halves, each = one joint chunk
            # pair (re|im interleaved); ScalarE does t1 = s1*p, t2 = c1*p
            # from PSUM; VectorE does the +- adds and stage-0 muls.
            #   q1 within half h: y[2h] = t2 -+ t1(other), signs per chunk
            #     [-,+,+,-]; q0 across halves: pairs (0,3),(1,2), [-,-,+,+]
            with tc.tile_pool(name="pmm", bufs=2,
                              space=bass.MemorySpace.PSUM) as pmm_pool:
                JW = 2 * NW                       # joint chunk width (1024)
                for t in range(NTILES):
                    c0_ap = cos_sb[:, t * N:t * N + 1]
                    s0_ap = sin_sb[:, t * N:t * N + 1]
                    c1_ap = cos_sb[:, t * N + 1:t * N + 2]
                    s1_ap = sin_sb[:, t * N + 1:t * N + 2]
                    y = y_pool.tile([128, 2 * DIM], BF16, tag="y")
                    for h in range(2):
                        pm = pmm_pool.tile([128, 2 * JW], F32)
                        for k in range(KT):
                            for jc in range(2):
                                for n in range(2):
                                    off = jc * JW + n * NW
                                    nc.tensor.matmul(
                                        pm[:, off:off + NW],
                                        wts[t][:, k * 128:(k + 1) * 128],
                                        uts[2 * h + jc][
                                            :, k * JW + n * NW:
                                            k * JW + (n + 1) * NW],
                                        start=(k == 0), stop=(k == KT - 1))
                        t1 = tmp_pool.tile([128, 2 * JW], BF16, tag="t1")
                        t2 = tmp_pool.tile([128, 2 * JW], BF16, tag="t2")
                        nc.scalar.mul(t1[:], pm[:], s1_ap)
                        nc.scalar.mul(t2[:], pm[:], c1_ap)
                        opA, opB = (SUB, ADD) if h == 0 else (ADD, SUB)
                        nc.vector.tensor_tensor(
                            y[:, 2 * JW * h:2 * JW * h + JW],
                            t2[:, 0:JW], t1[:, JW:2 * JW], opA)
                        nc.vector.tensor_tensor(
                            y[:, 2 * JW * h + JW:2 * JW * (h + 1)],
                            t2[:, JW:2 * JW], t1[:, 0:JW], opB)
                    t1p = tmp_pool.tile([128, 2 * DIM], BF16, tag="t1p")
                    t2p = tmp_pool.tile([128, 2 * DIM], BF16, tag="t2p")
                    nc.vector.tensor_scalar_mul(t1p[:], y[:], s0_ap)
                    nc.vector.tensor_scalar_mul(t2p[:], y[:], c0_ap)
                    ot = out_pool.tile([128, 2 * DIM], BF16, tag="o")
                    for c, (pair, op) in enumerate(
                            ((3, SUB), (2, SUB), (1, ADD), (0, ADD))):
                        nc.vector.tensor_tensor(
                            ot[:, c * JW:(c + 1) * JW],
                            t2p[:, c * JW:(c + 1) * JW],
                            t1p[:, pair * JW:(pair + 1) * JW], op)
                    nc.sync.dma_start(out_d[t * 128:(t + 1) * 128, :], ot[:])
    nc.finalize()
    return nc

# ----------------------------------------------------------------- driver

_CACHE = {}


def make_inputs(X, params):
    X = np.ascontiguousarray(np.asarray(X, dtype=np.float32))
    params = np.asarray(params, dtype=np.float32)
    U = build_u_matrix(params)
    u_bf = np.ascontiguousarray(
        U.reshape(KT, 128, 2 * DIM).astype(ml_dtypes.bfloat16))
    ident = np.eye(128, dtype=ml_dtypes.bfloat16)
    return [{"x": X[c * BSH:(c + 1) * BSH], "u": u_bf, "ident": ident}
            for c in range(NCORES)]


def postprocess(results):
    flat = np.concatenate([results[c]["out"] for c in range(NCORES)],
                          axis=0).astype(np.float32)
    out = np.empty((BATCH, DIM, 2), dtype=np.float32)
    for c in range(4):
        re = flat[:, 1024 * c:1024 * c + 512]
        im = flat[:, 1024 * c + 512:1024 * (c + 1)]
        if c in (1, 3):
            re, im = re[:, ::-1], im[:, ::-1]
        out[:, 512 * c:512 * (c + 1), 0] = re
        out[:, 512 * c:512 * (c + 1), 1] = im
    return out


def kernel(X, params):
    if "nc" not in _CACHE:
        _CACHE["nc"] = build_kernel()
    nc = _CACHE["nc"]
    in_maps = make_inputs(X, params)
    res = run_bass_kernel_spmd(nc, in_maps, list(range(NCORES)))
    return postprocess(res.results)
